# revision 14
# baseline (speedup 1.0000x reference)
"""Trainium2 Bass kernel for BilinearInteraction.

out[b, p, :] = (x[b, i_p, :] @ W[p]) * x[b, j_p, :]  for pairs p=(i,j), i<j
B=4096, F=32, D=64, P=496.  Output 520MB -> output-bandwidth bound
(~216us/core floor at 358GB/s HBM).

Strategy:
 - Data parallel over batch: 8 cores x 512 rows.
 - Per core: 4 batch tiles of 128 rows (SBUF partition dim).
 - x tile [128, 2048] loaded naturally; features transposed on PE
   (2 features per [128,128] identity-matmul transpose) to get xT[d, b].
 - For feature block i, pairs (i, i+1..31) occupy contiguous output
   columns; fp32 matmul xT_i.T @ W_block streams W columns; even-i
   blocks use PE rows 0-63, odd-i rows 64-127 (also balances the W
   stack across SBUF partition halves).
 - DVE multiplies PSUM by x[:, j*64:...] (naturally contiguous per
   block) into [128, 4096] SBUF windows; 2MB DMAs stream out.
 - DMA traffic split across queues: out stores on sync (SP HWDGE),
   x loads on scalar (ACT HWDGE), W loads on gpsimd (SWDGE) so input
   loads never serialize behind each other or the output stream.
 - Next batch-tile's transposes are interleaved into the current
   tile's matmul stream so PE never has a transpose clump / idle gap.
"""

import numpy as np

B, F, D = 4096, 32, 64
P = F * (F - 1) // 2            # 496
NCORES = 8
BLOC = B // NCORES              # 512
BT = 128                        # batch tile rows
NBT = BLOC // BT                # 4
NT = F // 2                     # transposes per batch tile (16)
TOTCOL = P * D                  # 31744
WIN = 4096                      # output SBUF window columns
MM = 512                        # max fp32 matmul free dim (one PSUM bank)
TAILSPLIT = 1024                # last-window DMA split size


def _p0(i):
    return i * (F - 1) - i * (i - 1) // 2


def _blocks():
    """(i, gs, ge, parity_offset) per feature block, in i order."""
    out = []
    off = {0: 0, 1: 0}
    for i in range(F - 1):
        gs = _p0(i) * D
        w = (F - 1 - i) * D
        out.append((i, gs, gs + w, off[i % 2]))
        off[i % 2] += w
    return out


BLOCKS = _blocks()
W_EVEN_COLS = sum(ge - gs for i, gs, ge, _ in BLOCKS if i % 2 == 0)   # 16384
W_ODD_COLS = sum(ge - gs for i, gs, ge, _ in BLOCKS if i % 2 == 1)    # 15360


def _chunks(block):
    """Split block into matmul chunks that never cross a global 512 grid
    line (hence never cross a WIN boundary and fit one PSUM bank)."""
    i, gs, ge, po = block
    out = []
    g = gs
    while g < ge:
        g1 = min(ge, (g // MM + 1) * MM)
        out.append((i, g, g1, po + (g - gs)))
        g = g1
    return out


def _groups():
    """Per pair-group g: interleaved (even, odd) block chunks so the PE
    row halves alternate."""
    groups = []
    for k in range(0, F - 1, 2):
        a = _chunks(BLOCKS[k])
        b = _chunks(BLOCKS[k + 1]) if k + 1 < F - 1 else []
        merged = []
        for t in range(max(len(a), len(b))):
            if t < len(a):
                merged.append(a[t])
            if t < len(b):
                merged.append(b[t])
        groups.append(merged)
    return groups


GROUPS = _groups()
CHUNKS = [c for grp in GROUPS for c in grp]


def _dma_ranges(last_bt):
    """Output DMA column ranges for one batch tile; the very last window
    of the last batch tile is split for a shorter kernel tail."""
    ranges = []
    c = 0
    while c < TOTCOL:
        c1 = min(c + WIN, TOTCOL)
        if last_bt and c1 == TOTCOL:
            while c < TOTCOL:
                ranges.append((c, min(c + TAILSPLIT, TOTCOL)))
                c += TAILSPLIT
        else:
            ranges.append((c, c1))
        c = c1
    return ranges


def _range_emit_map(ranges):
    """range -> index of last chunk (in CHUNKS order) writing into it."""
    emit = {}
    for ci, (i, g0, g1, wo) in enumerate(CHUNKS):
        for r in ranges:
            if g0 < r[1] and g1 > r[0]:
                emit[r] = ci
    by_chunk = {}
    for r, ci in emit.items():
        by_chunk.setdefault(ci, []).append(r)
    return by_chunk


def build_bass():
    import concourse.bacc as bacc
    import concourse.mybir as mybir
    from concourse import tile

    fp32 = mybir.dt.float32
    nc = bacc.Bacc("TRN2", target_bir_lowering=False, debug=False)

    x_dram = nc.dram_tensor("x", [BLOC, F * D], fp32, kind="ExternalInput")
    we_dram = nc.dram_tensor("w_even", [D, W_EVEN_COLS], fp32, kind="ExternalInput")
    wo_dram = nc.dram_tensor("w_odd", [D, W_ODD_COLS], fp32, kind="ExternalInput")
    id_dram = nc.dram_tensor("ident", [BT, BT], fp32, kind="ExternalInput")
    out_dram = nc.dram_tensor("out", [BLOC, TOTCOL], fp32, kind="ExternalOutput")

    with tile.TileContext(nc) as tc:
        with (
            tc.tile_pool(name="const", bufs=1) as const_pool,
            tc.tile_pool(name="x", bufs=2) as x_pool,
            tc.tile_pool(name="xt", bufs=2) as xt_pool,
            tc.tile_pool(name="outw", bufs=4) as out_pool,
            tc.tile_pool(name="pmm", bufs=5, space="PSUM") as pmm_pool,
            tc.tile_pool(name="ptr", bufs=2, space="PSUM") as ptr_pool,
        ):
            ident = const_pool.tile([BT, BT], fp32, tag="ident")
            nc.scalar.dma_start(ident[:], id_dram[:])

            x_tiles = [None] * NBT
            xt_tiles = [None] * NBT
            # x0 rides the sync ring, which is otherwise idle until the
            # first output store (~15us) — gets the PE started earliest.
            x_tiles[0] = x_pool.tile([BT, F * D], fp32, tag="x", name="x_0")
            nc.sync.dma_start(x_tiles[0][:], x_dram[0:BT, :])

            # W per-block loads in consumption order on the SWDGE
            # (gpsimd) queue so they never delay x loads or out stores.
            w_sb = const_pool.tile([128, W_EVEN_COLS], fp32, tag="w")
            for i, gs, ge, po in BLOCKS:
                row = slice(0, 64) if i % 2 == 0 else slice(64, 128)
                dram = we_dram if i % 2 == 0 else wo_dram
                nc.gpsimd.dma_start(w_sb[row, po:po + ge - gs], dram[:, po:po + ge - gs])

            def emit_transpose(bt, t):
                ptr = ptr_pool.tile([BT, BT], fp32, tag="ptr", name=f"ptr_{bt}_{t}")
                nc.tensor.transpose(
                    ptr[:], x_tiles[bt][:, t * BT:(t + 1) * BT], ident[:]
                )
                nc.scalar.copy(xt_tiles[bt][:, t * BT:(t + 1) * BT], ptr[:])

            for bt in range(NBT):
                rows = slice(bt * BT, (bt + 1) * BT)
                if bt == 0:
                    xt_tiles[0] = xt_pool.tile([BT, NT * BT], fp32, tag="xt",
                                               name="xt_0")
                    for t in range(NT):
                        emit_transpose(0, t)
                nxt = bt + 1
                if nxt < NBT:
                    x_tiles[nxt] = x_pool.tile([BT, F * D], fp32, tag="x",
                                               name=f"x_{nxt}")
                    nc.scalar.dma_start(
                        x_tiles[nxt][:], x_dram[nxt * BT:(nxt + 1) * BT, :]
                    )
                    xt_tiles[nxt] = xt_pool.tile([BT, NT * BT], fp32, tag="xt",
                                                 name=f"xt_{nxt}")

                ranges = _dma_ranges(bt == NBT - 1)
                emit_after = _range_emit_map(ranges)
                win_tiles = {}
                x_sb = x_tiles[bt]
                xt_sb = xt_tiles[bt]

                ci = 0
                for g, grp in enumerate(GROUPS):
                    for (i, g0, g1, wo) in grp:
                        csize = g1 - g0
                        par = i % 2
                        prows = slice(0, 64) if par == 0 else slice(64, 128)
                        tpos = (0, 0) if par == 0 else (64, 0)
                        t = i // 2
                        lhsT = xt_sb[prows, t * BT:(t + 1) * BT]
                        rhs = w_sb[prows, wo:wo + csize]

                        pmm = pmm_pool.tile([BT, MM], fp32, tag="pmm",
                                            name=f"pmm_{bt}_{ci}")
                        nc.tensor.matmul(
                            pmm[:, 0:csize], lhsT, rhs,
                            start=True, stop=True, tile_position=tpos,
                        )

                        k = g0 // WIN
                        if k not in win_tiles:
                            win_tiles[k] = out_pool.tile(
                                [BT, WIN], fp32, tag="win", name=f"win_{bt}_{k}"
                            )
                        wt = win_tiles[k]
                        l0 = g0 - k * WIN
                        xoff = (i + 1) * D + (g0 - _p0(i) * D)
                        nc.vector.tensor_mul(
                            wt[:, l0:l0 + csize],
                            pmm[:, 0:csize],
                            x_sb[:, xoff:xoff + csize],
                        )

                        for (c0, c1) in emit_after.get(ci, ()):
                            k2 = c0 // WIN
                            l = c0 - k2 * WIN
                            nc.sync.dma_start(
                                out_dram[rows, c0:c1],
                                win_tiles[k2][:, l:l + (c1 - c0)],
                            )
                        ci += 1

                    # splice next tile's transposes into this matmul stream
                    if nxt < NBT and g >= 1:
                        emit_transpose(nxt, g - 1)
                if nxt < NBT:
                    emit_transpose(nxt, NT - 1)

    nc.compile()
    return nc


_CACHE = {}


def _get_nc():
    if "nc" not in _CACHE:
        _CACHE["nc"] = build_bass()
    return _CACHE["nc"]


def kernel(inputs, W):
    from concourse import bass_utils

    x = np.asarray(inputs, dtype=np.float32).reshape(B, F * D)
    Wt = np.ascontiguousarray(
        np.asarray(W, dtype=np.float32).transpose(1, 0, 2)
    ).reshape(D, TOTCOL)
    w_even = np.ascontiguousarray(
        np.concatenate([Wt[:, gs:ge] for i, gs, ge, _ in BLOCKS if i % 2 == 0], axis=1)
    )
    w_odd = np.ascontiguousarray(
        np.concatenate([Wt[:, gs:ge] for i, gs, ge, _ in BLOCKS if i % 2 == 1], axis=1)
    )
    ident = np.eye(BT, dtype=np.float32)

    in_maps = [
        {
            "x": np.ascontiguousarray(x[c * BLOC:(c + 1) * BLOC]),
            "w_even": w_even,
            "w_odd": w_odd,
            "ident": ident,
        }
        for c in range(NCORES)
    ]
    nc = _get_nc()
    res = bass_utils.run_bass_kernel_spmd(nc, in_maps, core_ids=list(range(NCORES)))
    out = np.concatenate([res.results[c]["out"] for c in range(NCORES)], axis=0)
    return out.reshape(B, P, D)
